# revision 14
# baseline (speedup 1.0000x reference)
"""Trainium2 Bass kernel for nn_DistanceLoss (retrieval_knn, 5-way 5-shot).

Computation (per reference):
    q  = relu(queries.flat @ W.T + b)          [5600, 1024]
    se = relu(support.flat @ W.T + b)          [1400, 1024]
    d2 = q_sq + s_sq - 2 q @ se.T              [5600, 1400]
    out[q, c] = -mean_t min_{j in class c} sqrt(relu(d2))

Sharding (8 cores):
  - data-parallel over queries: 13 queries (728 rows) per core (padded 100->104)
  - support projection sharded by support rows (175 rows/core), AllGathered
  - min over class = contiguous 280-col chunk (support rows class-sorted on host)

Layout: "transposed activations" — all matmul operands keep the contraction
dim on partitions. Host pre-transposes Q/W/S (free: happens outside the
device kernel). Bias is folded in as an extra contraction row; q_sq/s_sq are
folded into the distance matmul as two extra contraction rows, so PSUM holds
d2 directly. min(sqrt(relu(x))) == sqrt(relu(min(x))), so the min reduction
runs on raw d2 and sqrt touches only [rows, 5].

Matmuls run as float32r (full fp32 data, full-rate PE mode for N>=256).
"""

import sys

if "/opt/trn_rl_repo" not in sys.path:
    sys.path.insert(0, "/opt/trn_rl_repo")

import os

import numpy as np

import concourse.bacc as bacc
import concourse.mybir as mybir
import concourse.tile as tile
from concourse.bass_utils import run_bass_kernel_spmd

WAY, SHOT, T = 5, 5, 56
D_IN, D_OUT = 6144, 1024
N_Q, N_S = 100, 25
N_CORES = 8
QPC = 13                 # queries per core (104 padded)
RPC = QPC * T            # 728 query rows per core
NQR = N_CORES * RPC      # 5824 padded query rows
NSR = N_S * T            # 1400 support rows
SPC = NSR // N_CORES     # 175 support rows per core
KT = D_IN // 128         # 48 full k-tiles
GK = 8                   # k-tiles per group
NGROUPS = KT // GK       # 6 groups
NCH = RPC // 2           # 364: query-row matmul chunk
CLS = NSR // WAY         # 280 columns per class
MT = (RPC + 127) // 128  # 6 row tiles (5x128 + 88)
SMW = (128, SPC - 128)   # support row-tile widths (128, 47)
SPCP = 176               # SPC padded so allgather payload is 64B-multiple

f32 = mybir.dt.float32
f32r = mybir.dt.float32r
AF = mybir.ActivationFunctionType
ALU = mybir.AluOpType
AX = mybir.AxisListType


_MODE = os.environ.get("KERNEL_MODE", "full")
_DBG_NGROUPS = int(os.environ.get("DBG_NGROUPS", "0"))   # 0 = all
_DBG_NO_EPI = os.environ.get("DBG_NO_EPI", "0") == "1"


def _build_nc():
    nc = bacc.Bacc("TRN2", target_bir_lowering=False, debug=False,
                   num_devices=N_CORES)
    qT = nc.dram_tensor("qT", [D_IN + 1, RPC], f32, kind="ExternalInput")
    wT = nc.dram_tensor("wT", [D_IN + 1, D_OUT], f32, kind="ExternalInput")
    sT = nc.dram_tensor("sT", [D_IN + 1, SPC], f32, kind="ExternalInput")
    mmask = nc.dram_tensor("mmask", [MT * 128, QPC], f32, kind="ExternalInput")
    ident = nc.dram_tensor("ident", [128, 128], f32, kind="ExternalInput")
    onesd = nc.dram_tensor("onesd", [128, NSR], f32, kind="ExternalInput")
    out = nc.dram_tensor("out", [QPC, WAY], f32, kind="ExternalOutput")

    with tile.TileContext(nc) as tc:
        _body(tc, nc, qT, wT, sT, mmask, ident, onesd, out)
    nc.finalize()
    return nc


def _body(tc, nc, qT, wT, sT, mmask, ident, onesd, out):
    persist_ctx = tc.tile_pool(name="persist", bufs=1)
    persist = persist_ctx.__enter__()

    def ptile(shape, name):
        return persist.tile(shape, f32, tag=name, name=name)

    need_epi = not _DBG_NO_EPI
    need_post = _MODE != "phase1"

    # ---- persistent tiles (live across phases) ----
    qacc = [persist.tile([128, RPC], f32r, tag=f"qacc{m}", name=f"qacc{m}")
            for m in range(8)]
    sacc = [ptile([SMW[sm], D_OUT], f"sacc{sm}") for sm in range(2)]
    ssq_cols = None
    if need_epi:
        ssq_cols = (ptile([128, 1], "ssq0"), ptile([SMW[1], 1], "ssq1"))
    if need_post:
        ident_t = ptile([128, 128], "ident_t")
        nc.sync.dma_start(out=ident_t[:], in_=ident[:])
        ones_col = persist.tile([128, 1], f32r, tag="ones_col", name="ones_col")
        nc.sync.dma_start(out=ones_col[:], in_=onesd[:, 0:1].bitcast(f32r))
        qsq_row = ptile([1, RPC], "qsq_row")
        ones_row = persist.tile([1, NSR], f32r, tag="ones_row", name="ones_row")
        nc.sync.dma_start(out=ones_row[:], in_=onesd[0:1, :].bitcast(f32r))
        ssq_full = persist.tile([1, NSR], f32r, tag="ssq_full", name="ssq_full")
        ssq_row = ptile([1, SPC], "ssq_row")
        mins = [ptile([128, WAY], f"mins{mt}") for mt in range(MT)]

    # ragged contraction row (bias / ones): loaded once
    wr = persist.tile([1, D_OUT], f32r, tag="wr", name="wr")
    qr = persist.tile([1, RPC], f32r, tag="qr", name="qr")
    sr = persist.tile([1, SPC], f32r, tag="sr", name="sr")
    nc.sync.dma_start(out=wr[:], in_=wT[D_IN:D_IN + 1, :].bitcast(f32r))
    nc.sync.dma_start(out=qr[:], in_=qT[D_IN:D_IN + 1, :].bitcast(f32r))
    nc.sync.dma_start(out=sr[:], in_=sT[D_IN:D_IN + 1, :].bitcast(f32r))

    # ---- phase 1: projections, k-grouped; W tiles shared by both ----
    with (
        tc.tile_pool(name="wpool", bufs=2 * GK) as wpool,
        tc.tile_pool(name="qpool", bufs=2 * GK) as qpool,
        tc.tile_pool(name="spool", bufs=2 * GK) as spool,
        tc.tile_pool(name="pq", bufs=4, space="PSUM") as pqpool,
        tc.tile_pool(name="ps", bufs=4, space="PSUM") as pspool,
        tc.tile_pool(name="ssq_scratch", bufs=2) as scratch_pool,
    ):
        ngroups = _DBG_NGROUPS or NGROUPS
        for g in range(ngroups):
            kts = list(range(g * GK, (g + 1) * GK))
            last = g == ngroups - 1
            wt, st, qt = {}, {}, {}
            for kt in kts:
                wt[kt] = wpool.tile([128, D_OUT], f32r, tag="w", name=f"w{kt}")
                nc.sync.dma_start(out=wt[kt][:],
                                  in_=wT[kt * 128:(kt + 1) * 128, :]
                                  .bitcast(f32r))
                st[kt] = spool.tile([128, SPC], f32r, tag="s", name=f"s{kt}")
                nc.sync.dma_start(out=st[kt][:],
                                  in_=sT[kt * 128:(kt + 1) * 128, :]
                                  .bitcast(f32r))
                qt[kt] = qpool.tile([128, RPC], f32r, tag="q", name=f"q{kt}")
                nc.sync.dma_start(out=qt[kt][:],
                                  in_=qT[kt * 128:(kt + 1) * 128, :]
                                  .bitcast(f32r))

            # support projection: se[srows, dout] += (S.T-tile).T @ W-tile
            for sm in range(2):
                mw = SMW[sm]
                msl = slice(sm * 128, sm * 128 + mw)
                for n in range(2):
                    nsl = slice(n * 512, (n + 1) * 512)
                    pst = pspool.tile([128, 512], f32, tag="ps", name="pst")
                    for i, kt in enumerate(kts):
                        nc.tensor.matmul(
                            pst[:mw, :],
                            st[kt][:, msl],
                            wt[kt][:, nsl],
                            start=(i == 0),
                            stop=(i == GK - 1 and not last),
                        )
                    if last:
                        nc.tensor.matmul(
                            pst[:mw, :],
                            sr[:, msl],
                            wr[:, nsl],
                            start=False, stop=True,
                        )
                    if g == 0:
                        nc.vector.tensor_copy(sacc[sm][:, nsl], pst[:mw, :])
                    else:
                        nc.vector.tensor_add(sacc[sm][:, nsl],
                                             sacc[sm][:, nsl], pst[:mw, :])

            if last and not _DBG_NO_EPI:
                # support epilogue: sacc = -2*relu(raw) = min(-2*raw, 0);
                # s_sq = sum(relu(raw)^2) = sum((0.5*sacc)^2) via ACT accum
                for sm in range(2):
                    mw = SMW[sm]
                    nc.vector.tensor_scalar(sacc[sm][:], sacc[sm][:],
                                            -2.0, 0.0, ALU.mult, ALU.min)
                    sc = scratch_pool.tile([128, D_OUT], f32, tag="ssq_sc", name="ssq_sc")
                    nc.scalar.activation(sc[:mw, :], sacc[sm][:], AF.Square,
                                         scale=0.5,
                                         accum_out=ssq_cols[sm][:mw, :])

            # query projection: q.T[dout, rows] += W-tile.T @ Q.T-tile
            for m in range(8):
                msl = slice(m * 128, (m + 1) * 128)
                for n in range(2):
                    nsl = slice(n * NCH, (n + 1) * NCH)
                    pqt = pqpool.tile([128, NCH], f32, tag="pq", name="pqt")
                    for i, kt in enumerate(kts):
                        nc.tensor.matmul(
                            pqt[:],
                            wt[kt][:, msl],
                            qt[kt][:, nsl],
                            start=(i == 0),
                            stop=(i == GK - 1 and not last),
                        )
                    if last:
                        nc.tensor.matmul(
                            pqt[:],
                            wr[:, msl],
                            qr[:, nsl],
                            start=False, stop=True,
                        )
                    if g == 0:
                        nc.vector.tensor_copy(qacc[m][:, nsl], pqt[:])
                    else:
                        nc.vector.tensor_add(qacc[m][:, nsl],
                                             qacc[m][:, nsl], pqt[:])

    if _MODE == "phase1":
        with tc.tile_pool(name="outs_dbg", bufs=1) as outs_pool:
            out_s = outs_pool.tile([QPC, WAY], f32, tag="out_s", name="out_s")
            nc.vector.tensor_copy(out_s[:], qacc[0][:QPC, :WAY].bitcast(f32))
            nc.sync.dma_start(out=out[:], in_=out_s[:])
        persist_ctx.__exit__(None, None, None)
        return

    # ---- allgather se.T parts + s_sq (early: unblocks phase 2) ----
    dram_ctx = tc.tile_pool(name="dram", bufs=1, space="DRAM")
    dram = dram_ctx.__enter__()
    ag_in = dram.tile([D_OUT + 1, SPCP], f32, tag="ag_in", name="ag_in")
    ag_out = dram.tile([N_CORES, D_OUT + 1, SPCP], f32, tag="ag_out", name="ag_out",
                       addr_space="Local" if _MODE == "nocc" else "Shared")
    with tc.tile_pool(name="setl", bufs=1) as setl_pool, \
         tc.tile_pool(name="ptr", bufs=4, space="PSUM") as ptr_pool:
        for j in range(8):
            setl = setl_pool.tile([128, SPC], f32, tag=f"setl{j}", name=f"setl{j}")
            for sm in range(2):
                mw = SMW[sm]
                ptr = ptr_pool.tile([128, 128], f32, tag="ptr", name="ptr")
                nc.tensor.transpose(
                    ptr[:, :mw],
                    sacc[sm][:, j * 128:(j + 1) * 128],
                    ident_t[:mw, :mw],
                )
                nc.vector.tensor_copy(setl[:, sm * 128:sm * 128 + mw],
                                      ptr[:, :mw])
            nc.sync.dma_start(out=ag_in[j * 128:(j + 1) * 128, 0:SPC],
                              in_=setl[:])
        # s_sq row: transpose [mw,1] -> [1,mw]
        for sm in range(2):
            mw = SMW[sm]
            ptr = ptr_pool.tile([128, 128], f32, tag="ptr", name="ptr")
            nc.tensor.transpose(ptr[:1, :mw], ssq_cols[sm][:mw, :],
                                ident_t[:mw, :mw])
            nc.vector.tensor_copy(ssq_row[:, sm * 128:sm * 128 + mw],
                                  ptr[:1, :mw])
        nc.sync.dma_start(out=ag_in[D_OUT:D_OUT + 1, 0:SPC], in_=ssq_row[:])

    if _MODE == "nocc":
        for c in range(N_CORES):
            nc.sync.dma_start(out=ag_out[c], in_=ag_in[:])
    else:
        nc.gpsimd.collective_compute(
            "AllGather",
            ALU.bypass,
            replica_groups=[list(range(N_CORES))],
            ins=[ag_in[:]],
            outs=[ag_out[:]],
        )

    # ---- query epilogue: relu, q_sq (overlaps with the collective) ----
    with (
        tc.tile_pool(name="sqpool", bufs=2) as sqpool,
        tc.tile_pool(name="pqsq", bufs=2, space="PSUM") as pqsqpool,
    ):
        pqsq = [pqsqpool.tile([1, NCH], f32, tag="pqsq", name=f"pqsq{n}") for n in range(2)]
        for m in range(8 if not _DBG_NO_EPI else 0):
            nc.vector.tensor_scalar_max(qacc[m][:], qacc[m][:], 0.0)
            sq = sqpool.tile([128, RPC], f32, tag="sq", name="sq")
            nc.scalar.activation(sq[:].bitcast(f32r), qacc[m][:], AF.Square)
            for n in range(2):
                nsl = slice(n * NCH, (n + 1) * NCH)
                nc.tensor.matmul(
                    pqsq[n][:],
                    ones_col[:],
                    sq[:, nsl].bitcast(f32r),
                    start=(m == 0), stop=(m == 7),
                )
        for n in range(2 if not _DBG_NO_EPI else 0):
            nsl = slice(n * NCH, (n + 1) * NCH)
            nc.vector.tensor_copy(qsq_row[0:1, nsl].bitcast(f32r), pqsq[n][:])

    # ---- phase 2: distance + per-class min + mean ----
    with (
        tc.tile_pool(name="seTp", bufs=1) as seT_pool,
        tc.tile_pool(name="mk", bufs=1) as mk_pool,
        tc.tile_pool(name="pd", bufs=6, space="PSUM") as pd_pool,
        tc.tile_pool(name="po", bufs=1, space="PSUM") as po_pool,
        tc.tile_pool(name="outs", bufs=1) as outs_pool,
    ):
        seT = []
        for j in range(8):
            t_ = seT_pool.tile([128, NSR], f32r, tag=f"seT{j}", name=f"seT{j}")
            seT.append(t_)
            for c in range(N_CORES):
                nc.sync.dma_start(
                    out=t_[:, c * SPC:(c + 1) * SPC],
                    in_=ag_out[c, j * 128:(j + 1) * 128, 0:SPC]
                    .bitcast(f32r))
        for c in range(N_CORES):
            nc.sync.dma_start(out=ssq_full[0:1, c * SPC:(c + 1) * SPC],
                              in_=ag_out[c, D_OUT:D_OUT + 1, 0:SPC]
                              .bitcast(f32r))

        for mt in range(MT):
            nc.vector.memset(mins[mt][:], 0.0)

        mkt = []
        for mt in range(MT):
            t_ = mk_pool.tile([128, QPC], f32, tag=f"mk{mt}", name=f"mk{mt}")
            mkt.append(t_)
            nc.sync.dma_start(out=t_[:], in_=mmask[mt * 128:(mt + 1) * 128, :])

        for mt in range(MT):
            mw = min(128, RPC - mt * 128)
            msl = slice(mt * 128, mt * 128 + mw)
            for ch in range(WAY):
                nsl = slice(ch * CLS, (ch + 1) * CLS)
                pd = pd_pool.tile([128, CLS], f32, tag="pd", name="pd")
                for j in range(8):
                    nc.tensor.matmul(
                        pd[:mw, :],
                        qacc[j][:, msl],
                        seT[j][:, nsl],
                        start=(j == 0), stop=False,
                    )
                nc.tensor.matmul(
                    pd[:mw, :],
                    qsq_row[:, msl].bitcast(f32r),
                    ones_row[:, nsl],
                    start=False, stop=False,
                )
                nc.tensor.matmul(
                    pd[:mw, :],
                    ones_row[:, msl],
                    ssq_full[:, nsl],
                    start=False, stop=True,
                )
                nc.vector.tensor_reduce(
                    mins[mt][:mw, ch:ch + 1], pd[:mw, :],
                    axis=AX.X, op=ALU.min)
            nc.vector.tensor_scalar_max(mins[mt][:], mins[mt][:], 0.0)
            nc.scalar.activation(mins[mt][:], mins[mt][:], AF.Sqrt)

        po = po_pool.tile([QPC, WAY], f32, tag="po", name="po")
        for mt in range(MT):
            nc.tensor.matmul(po[:], mkt[mt][:], mins[mt][:],
                             start=(mt == 0), stop=(mt == MT - 1))
        out_s = outs_pool.tile([QPC, WAY], f32, tag="out_s", name="out_s")
        nc.vector.tensor_copy(out_s[:], po[:])
        nc.sync.dma_start(out=out[:], in_=out_s[:])

    dram_ctx.__exit__(None, None, None)
    persist_ctx.__exit__(None, None, None)


_NC_CACHE = {}


def _get_nc():
    if "nc" not in _NC_CACHE:
        _NC_CACHE["nc"] = _build_nc()
    return _NC_CACHE["nc"]


def make_in_maps(support_set, support_labels, queries, clsW_w, clsW_b):
    support_set = np.asarray(support_set, dtype=np.float32)
    support_labels = np.asarray(support_labels)
    queries = np.asarray(queries, dtype=np.float32)
    clsW_w = np.asarray(clsW_w, dtype=np.float32)
    clsW_b = np.asarray(clsW_b, dtype=np.float32)

    # class-sort support rows so each class is a contiguous 280-column block
    perm = np.argsort(support_labels, kind="stable")
    S = support_set[perm].reshape(NSR, D_IN)

    STa = np.empty((D_IN + 1, NSR), np.float32)
    STa[:D_IN] = S.T
    STa[D_IN] = 1.0

    Qp = np.zeros((NQR, D_IN), np.float32)
    Qp[:N_Q * T] = queries.reshape(N_Q * T, D_IN)
    QTa = np.empty((D_IN + 1, NQR), np.float32)
    QTa[:D_IN] = Qp.T
    QTa[D_IN] = 1.0

    WTa = np.empty((D_IN + 1, D_OUT), np.float32)
    WTa[:D_IN] = clsW_w.T
    WTa[D_IN] = clsW_b

    mmask = np.zeros((MT * 128, QPC), np.float32)
    r = np.arange(RPC)
    mmask[r, r // T] = -1.0 / T

    ident = np.eye(128, dtype=np.float32)
    onesd = np.ones((128, NSR), np.float32)

    in_maps = []
    for c in range(N_CORES):
        in_maps.append({
            "qT": np.ascontiguousarray(QTa[:, c * RPC:(c + 1) * RPC]),
            "wT": WTa,
            "sT": np.ascontiguousarray(STa[:, c * SPC:(c + 1) * SPC]),
            "mmask": mmask,
            "ident": ident,
            "onesd": onesd,
        })
    return in_maps


def kernel(support_set, support_labels, queries, clsW_w, clsW_b):
    in_maps = make_in_maps(support_set, support_labels, queries, clsW_w,
                           clsW_b)
    nc = _get_nc()
    res = run_bass_kernel_spmd(nc, in_maps, list(range(N_CORES)))
    out = np.concatenate([res.results[c]["out"] for c in range(N_CORES)], 0)
    return np.ascontiguousarray(out[:N_Q]).astype(np.float32)
